# revision 23
# baseline (speedup 1.0000x reference)
"""Trainium2 Bass kernel for nn_ExistLCross (masked weighted -log loss).

reference:
    mask = (label == 1)
    per_elem = -log(pred + 0.01) * existmap * mask
    loss = einsum('nchw,c->', per_elem, Wl) / sum(label_sum)

Sharding: data-parallel over batch N=8 -> one batch item per NeuronCore.
Each core processes its [C=16, 512, 512] shard as 16 planes of [128, 2048]
(the last class in 4 quarter-plane chunks to shorten the post-DMA tail):
    ACT : logp = Ln(pred + 0.01)            (bias folded into activation)
    DVE : prod = logp * existmap            (tensor_tensor)
    DVE : scr  = (label == 1) * prod        (scalar_tensor_tensor)
          accum_out -> acc[:, col]  (per-partition sums, one col per chunk)
Each core DMAs acc [128, 19] back; the host applies the per-class weights
-Wl/sum(label_sum) and adds up the 8 per-core partials (the all-reduce).
"""

import sys
import types
from contextlib import ExitStack

import numpy as np

import concourse.bacc as bacc
import concourse.bass as bass
import concourse.tile as tile
from concourse import bass_utils, mybir

# This container's antenv lacks axon_hooks; bass_utils imports it whenever
# tracing is requested (e.g. via BASS_TRACE in the environment). Provide a
# no-op implementation so tracing degrades gracefully instead of raising.
if "antenv.axon_hooks" not in sys.modules:
    _hooks = types.ModuleType("antenv.axon_hooks")
    _hooks._hook = None
    _hooks.set_axon_ntff_profile_hook = lambda h: setattr(_hooks, "_hook", h)
    _hooks.get_axon_ntff_profile_hook = lambda: _hooks._hook
    sys.modules["antenv.axon_hooks"] = _hooks

N, C, H, W = 8, 16, 512, 512
P = 128
FREE = (H * W) // P  # 2048
EPS = 0.01
N_CORES = 8
NACC = C + 3  # last class spread over 4 accumulator columns

_nc_cache = []


def _build_nc() -> bass.Bass:
    nc = bacc.Bacc("TRN2", target_bir_lowering=False, debug=False,
                   num_devices=N_CORES)

    pred_d = nc.dram_tensor("pred", [C, P, FREE], mybir.dt.float32,
                            kind="ExternalInput").ap()
    lab_d = nc.dram_tensor("label", [C, P, FREE], mybir.dt.int32,
                           kind="ExternalInput").ap()
    em_d = nc.dram_tensor("existmap", [C, P, FREE], mybir.dt.float32,
                          kind="ExternalInput").ap()
    out_d = nc.dram_tensor("out", [P, NACC], mybir.dt.float32,
                           kind="ExternalOutput").ap()

    with tile.TileContext(nc) as tc, ExitStack() as ctx:
        ins = ctx.enter_context(tc.tile_pool(name="ins", bufs=4))
        work = ctx.enter_context(tc.tile_pool(name="work", bufs=2))
        singles = ctx.enter_context(tc.tile_pool(name="singles", bufs=1))

        acc = singles.tile([P, NACC], mybir.dt.float32)
        ones = singles.tile([P, 1], mybir.dt.float32)
        eps_t = singles.tile([P, 1], mybir.dt.float32)
        nc.vector.memset(ones, 1.0)
        # eps = ones*0 + EPS, produced on ACT so every Ln below depends on
        # it same-engine (no cross-engine wait, no pre-Tile barrier).
        nc.scalar.activation(eps_t, ones, mybir.ActivationFunctionType.Copy,
                             bias=EPS, scale=0.0)

        def compute(pred_v, lab_v, em_v, fr, acc_col):
            """log/mask chain on [P, fr] SBUF views, accumulating the
            per-partition sums into acc[:, acc_col]."""
            logp = work.tile([P, fr], mybir.dt.float32, tag="logp")
            nc.scalar.activation(logp, pred_v,
                                 mybir.ActivationFunctionType.Ln, bias=eps_t)

            prod = work.tile([P, fr], mybir.dt.float32, tag="prod")
            nc.vector.tensor_mul(prod, logp, em_v)

            scr = work.tile([P, fr], mybir.dt.float32, tag="scr")
            nc.vector.scalar_tensor_tensor(
                out=scr, in0=lab_v, scalar=1.0, in1=prod,
                op0=mybir.AluOpType.is_equal, op1=mybir.AluOpType.mult,
                accum_out=acc[:, acc_col:acc_col + 1],
            )

        def load(pred_ap, lab_ap, em_ap, fr):
            """One [P, fr] contiguous block of each input on the SP ring."""
            pred_t = ins.tile([P, fr], mybir.dt.float32, tag="pred")
            lab_t = ins.tile([P, fr], mybir.dt.int32, tag="lab")
            em_t = ins.tile([P, fr], mybir.dt.float32, tag="em")
            nc.sync.dma_start(out=pred_t, in_=pred_ap)
            nc.sync.dma_start(out=lab_t, in_=lab_ap)
            nc.sync.dma_start(out=em_t, in_=em_ap)
            return pred_t, lab_t, em_t

        for c in range(C - 1):
            tiles = load(pred_d[c], lab_d[c], em_d[c], FREE)
            compute(*tiles, FREE, c)

        # last class: 4 quarter chunks to shorten the post-DMA tail
        c = C - 1
        Q = FREE // 4
        for q in range(4):
            sl = slice(q * Q, (q + 1) * Q)
            tiles = load(pred_d[c, :, sl], lab_d[c, :, sl],
                         em_d[c, :, sl], Q)
            compute(*tiles, Q, C - 1 + q)

        nc.sync.dma_start(out=out_d, in_=acc)

    nc.compile()
    return nc


def _get_nc() -> bass.Bass:
    if not _nc_cache:
        _nc_cache.append(_build_nc())
    return _nc_cache[0]


def _run(pred, label, Wl, label_sum, existmap, **spmd_kwargs):
    pred = np.ascontiguousarray(np.asarray(pred, dtype=np.float32))
    label = np.ascontiguousarray(np.asarray(label, dtype=np.int32))
    Wl = np.asarray(Wl, dtype=np.float32)
    label_sum = np.asarray(label_sum, dtype=np.float32)
    existmap = np.ascontiguousarray(np.asarray(existmap, dtype=np.float32))

    denom = np.sum(label_sum, dtype=np.float32)
    wl_scaled = (-Wl / denom).astype(np.float32)
    # last class occupies 4 accumulator columns (quarter-plane chunks)
    wl_ext = np.concatenate([wl_scaled, np.repeat(wl_scaled[-1:], 3)])

    in_maps = []
    for i in range(N_CORES):
        in_maps.append({
            "pred": pred[i].reshape(C, P, FREE),
            "label": label[i].reshape(C, P, FREE),
            "existmap": existmap[i].reshape(C, P, FREE),
        })

    nc = _get_nc()
    res = bass_utils.run_bass_kernel_spmd(
        nc, in_maps, core_ids=list(range(N_CORES)), **spmd_kwargs)

    total = np.float32(0.0)
    for r in res.results:
        # [P, NACC] partition sums -> weighted scalar (f32 like reference)
        per_class = r["out"].sum(axis=0, dtype=np.float32)
        total += np.float32((per_class * wl_ext).sum(dtype=np.float32))
    return np.array(total, dtype=np.float32), res


def kernel(pred, label, Wl, label_sum, existmap):
    out, _ = _run(pred, label, Wl, label_sum, existmap)
    return out


# revision 26
# speedup vs baseline: 1.0712x; 1.0712x over previous
"""Trainium2 Bass kernel for nn_ExistLCross (masked weighted -log loss).

reference:
    mask = (label == 1)
    per_elem = -log(pred + 0.01) * existmap * mask
    loss = einsum('nchw,c->', per_elem, Wl) / sum(label_sum)

Sharding: data-parallel over batch N=8 -> one batch item per NeuronCore.
Each core views its [16, 512, 512] shard as 8 flat 2-class blocks of
[128, 4096] (16 KB contiguous per partition -> ideal 2 MiB DMAs). Within a
block, partitions 0..63 hold the even class and 64..127 the odd class, so
per-partition sums separate the classes for free. Per block:
    ACT : logp = Ln(pred + 0.01)               (bias via ACT-made eps tile)
    DVE : logp *= existmap                     (tensor_tensor, in place)
    DVE : logp = (label == 1) * logp           (scalar_tensor_tensor, in
          place) with accum_out -> acc[:, col] (per-partition sums)
The final block runs as 8 eighth-chunks to shorten the post-DMA tail.
Each core DMAs acc [128, 15] back; the host applies the per-class weights
-Wl/sum(label_sum) via a [128, 15] weight map and adds up the 8 per-core
partials (the all-reduce).
"""

import sys
import types
from contextlib import ExitStack

import numpy as np

import concourse.bacc as bacc
import concourse.bass as bass
import concourse.tile as tile
from concourse import bass_utils, mybir

# This container's antenv lacks axon_hooks; bass_utils imports it whenever
# tracing is requested (e.g. via BASS_TRACE in the environment). Provide a
# no-op implementation so tracing degrades gracefully instead of raising.
if "antenv.axon_hooks" not in sys.modules:
    _hooks = types.ModuleType("antenv.axon_hooks")
    _hooks._hook = None
    _hooks.set_axon_ntff_profile_hook = lambda h: setattr(_hooks, "_hook", h)
    _hooks.get_axon_ntff_profile_hook = lambda: _hooks._hook
    sys.modules["antenv.axon_hooks"] = _hooks

N, C, H, W = 8, 16, 512, 512
P = 128
B2 = C // 2          # 8 two-class blocks
BFREE = 2 * H * W // P  # 4096 elements per partition per block
EPS = 0.01
N_CORES = 8
NTAIL = 8            # last block in 8 chunks
NACC = (B2 - 1) + NTAIL  # 15 accumulator columns

_nc_cache = []


def _build_nc() -> bass.Bass:
    nc = bacc.Bacc("TRN2", target_bir_lowering=False, debug=False,
                   num_devices=N_CORES)

    pred_d = nc.dram_tensor("pred", [B2, P, BFREE], mybir.dt.float32,
                            kind="ExternalInput").ap()
    lab_d = nc.dram_tensor("label", [B2, P, BFREE], mybir.dt.int32,
                           kind="ExternalInput").ap()
    em_d = nc.dram_tensor("existmap", [B2, P, BFREE], mybir.dt.float32,
                          kind="ExternalInput").ap()
    out_d = nc.dram_tensor("out", [P, NACC], mybir.dt.float32,
                           kind="ExternalOutput").ap()

    with tile.TileContext(nc) as tc, ExitStack() as ctx:
        ins = ctx.enter_context(tc.tile_pool(name="ins", bufs=2))
        work = ctx.enter_context(tc.tile_pool(name="work", bufs=2))
        singles = ctx.enter_context(tc.tile_pool(name="singles", bufs=1))

        acc = singles.tile([P, NACC], mybir.dt.float32)
        ones = singles.tile([P, 1], mybir.dt.float32)
        eps_t = singles.tile([P, 1], mybir.dt.float32)
        nc.vector.memset(ones, 1.0)
        # eps = ones*0 + EPS, produced on ACT so every Ln below depends on
        # it same-engine (no cross-engine wait, no pre-Tile barrier).
        nc.scalar.activation(eps_t, ones, mybir.ActivationFunctionType.Copy,
                             bias=EPS, scale=0.0)

        def block(pred_ap, lab_ap, em_ap, fr, acc_col):
            """Load one [P, fr] flat block of each input and accumulate
            its masked-log per-partition sums into acc[:, acc_col]."""
            pred_t = ins.tile([P, fr], mybir.dt.float32, tag="pred")
            lab_t = ins.tile([P, fr], mybir.dt.int32, tag="lab")
            em_t = ins.tile([P, fr], mybir.dt.float32, tag="em")
            nc.sync.dma_start(out=pred_t, in_=pred_ap)
            nc.sync.dma_start(out=lab_t, in_=lab_ap)
            nc.sync.dma_start(out=em_t, in_=em_ap)

            logp = work.tile([P, fr], mybir.dt.float32, tag="logp")
            nc.scalar.activation(logp, pred_t,
                                 mybir.ActivationFunctionType.Ln, bias=eps_t)
            prod = work.tile([P, fr], mybir.dt.float32, tag="prod")
            nc.vector.tensor_mul(prod, logp, em_t)
            scr = work.tile([P, fr], mybir.dt.float32, tag="scr", bufs=1)
            nc.vector.scalar_tensor_tensor(
                out=scr, in0=lab_t, scalar=1.0, in1=prod,
                op0=mybir.AluOpType.is_equal, op1=mybir.AluOpType.mult,
                accum_out=acc[:, acc_col:acc_col + 1],
            )

        for b in range(B2 - 1):
            block(pred_d[b], lab_d[b], em_d[b], BFREE, b)

        # last block: 8 chunks to shorten the post-DMA tail
        b = B2 - 1
        Q = BFREE // NTAIL
        for q in range(NTAIL):
            sl = slice(q * Q, (q + 1) * Q)
            block(pred_d[b, :, sl], lab_d[b, :, sl], em_d[b, :, sl],
                  Q, B2 - 1 + q)

        nc.sync.dma_start(out=out_d, in_=acc)

    nc.compile()
    return nc


def _get_nc() -> bass.Bass:
    if not _nc_cache:
        _nc_cache.append(_build_nc())
    return _nc_cache[0]


def _weight_map(Wl, label_sum):
    """[P, NACC] weight map matching the flat-block accumulator layout:
    within any block column, partitions 0..63 hold the even class and
    64..127 the odd class."""
    denom = np.sum(label_sum, dtype=np.float32)
    wl = (-Wl / denom).astype(np.float32)
    cols = []
    for b in range(B2 - 1):
        cols.append(np.concatenate([np.full(P // 2, wl[2 * b]),
                                    np.full(P // 2, wl[2 * b + 1])]))
    tail = np.concatenate([np.full(P // 2, wl[C - 2]),
                           np.full(P // 2, wl[C - 1])])
    cols.extend([tail] * NTAIL)
    return np.stack(cols, axis=1).astype(np.float32)


def _run(pred, label, Wl, label_sum, existmap, **spmd_kwargs):
    pred = np.ascontiguousarray(np.asarray(pred, dtype=np.float32))
    label = np.ascontiguousarray(np.asarray(label, dtype=np.int32))
    Wl = np.asarray(Wl, dtype=np.float32)
    label_sum = np.asarray(label_sum, dtype=np.float32)
    existmap = np.ascontiguousarray(np.asarray(existmap, dtype=np.float32))

    wl_map = _weight_map(Wl, label_sum)

    in_maps = []
    for i in range(N_CORES):
        in_maps.append({
            "pred": pred[i].reshape(B2, P, BFREE),
            "label": label[i].reshape(B2, P, BFREE),
            "existmap": existmap[i].reshape(B2, P, BFREE),
        })

    nc = _get_nc()
    res = bass_utils.run_bass_kernel_spmd(
        nc, in_maps, core_ids=list(range(N_CORES)), **spmd_kwargs)

    total = np.float32(0.0)
    for r in res.results:
        total += np.float32((r["out"] * wl_map).sum(dtype=np.float32))
    return np.array(total, dtype=np.float32), res


def kernel(pred, label, Wl, label_sum, existmap):
    out, _ = _run(pred, label, Wl, label_sum, existmap)
    return out


# revision 29
# speedup vs baseline: 1.2197x; 1.1386x over previous
"""Trainium2 Bass kernel for nn_ExistLCross (masked weighted -log loss).

reference:
    mask = (label == 1)
    per_elem = -log(pred + 0.01) * existmap * mask
    loss = einsum('nchw,c->', per_elem, Wl) / sum(label_sum)

Sharding: data-parallel over batch N=8 -> one batch item per NeuronCore.
Each core views its [16, 512, 512] shard as 8 flat 2-class blocks of
[128, 4096] (16 KB contiguous per partition -> ideal 2 MiB DMAs). Within a
block, partitions 0..63 hold the even class and 64..127 the odd class, so
per-partition sums separate the classes for free. Per block:
    ACT : logp = Ln(pred + 0.01)               (bias via ACT-made eps tile)
    DVE : logp *= existmap                     (tensor_tensor, in place)
    DVE : logp = (label == 1) * logp           (scalar_tensor_tensor, in
          place) with accum_out -> acc[:, col] (per-partition sums)
The final block runs as 8 eighth-chunks to shorten the post-DMA tail.
Each core DMAs acc [128, 15] back; the host applies the per-class weights
-Wl/sum(label_sum) via a [128, 15] weight map and adds up the 8 per-core
partials (the all-reduce).
"""

import sys
import types
from contextlib import ExitStack

import numpy as np

import concourse.bacc as bacc
import concourse.bass as bass
import concourse.tile as tile
from concourse import bass_utils, mybir

# This container's antenv lacks axon_hooks; bass_utils imports it whenever
# tracing is requested (e.g. via BASS_TRACE in the environment). Provide a
# no-op implementation so tracing degrades gracefully instead of raising.
if "antenv.axon_hooks" not in sys.modules:
    _hooks = types.ModuleType("antenv.axon_hooks")
    _hooks._hook = None
    _hooks.set_axon_ntff_profile_hook = lambda h: setattr(_hooks, "_hook", h)
    _hooks.get_axon_ntff_profile_hook = lambda: _hooks._hook
    sys.modules["antenv.axon_hooks"] = _hooks

N, C, H, W = 8, 16, 512, 512
P = 128
B2 = C // 2          # 8 two-class blocks
BFREE = 2 * H * W // P  # 4096 elements per partition per block
HFREE = BFREE // 2
EPS = 0.01
N_CORES = 8
NTAIL = 4            # last block in 4 chunks
NACC = 2 * (B2 - 1) + NTAIL  # 18 accumulator columns (one per half-chain)

_nc_cache = []


def _build_nc() -> bass.Bass:
    nc = bacc.Bacc("TRN2", target_bir_lowering=False, debug=False,
                   num_devices=N_CORES)

    pred_d = nc.dram_tensor("pred", [B2, P, BFREE], mybir.dt.float32,
                            kind="ExternalInput").ap()
    lab_d = nc.dram_tensor("label", [B2, P, BFREE], mybir.dt.int32,
                           kind="ExternalInput").ap()
    em_d = nc.dram_tensor("existmap", [B2, P, BFREE], mybir.dt.float32,
                          kind="ExternalInput").ap()
    out_d = nc.dram_tensor("out", [P, NACC], mybir.dt.float32,
                           kind="ExternalOutput").ap()

    with tile.TileContext(nc) as tc, ExitStack() as ctx:
        ins = ctx.enter_context(tc.tile_pool(name="ins", bufs=2))
        work = ctx.enter_context(tc.tile_pool(name="work", bufs=2))
        singles = ctx.enter_context(tc.tile_pool(name="singles", bufs=1))

        acc = singles.tile([P, NACC], mybir.dt.float32)
        ones = singles.tile([P, 1], mybir.dt.float32)
        eps_t = singles.tile([P, 1], mybir.dt.float32)
        nc.vector.memset(ones, 1.0)
        # eps = ones*0 + EPS, produced on ACT so every Ln below depends on
        # it same-engine (no cross-engine wait, no pre-Tile barrier).
        nc.scalar.activation(eps_t, ones, mybir.ActivationFunctionType.Copy,
                             bias=EPS, scale=0.0)

        def compute(pred_v, lab_v, em_v, fr, acc_col):
            """log/mask chain on [P, fr] SBUF views, accumulating the
            per-partition sums into acc[:, acc_col]."""
            logp = work.tile([P, fr], mybir.dt.float32, tag="logp")
            nc.scalar.activation(logp, pred_v,
                                 mybir.ActivationFunctionType.Ln, bias=eps_t)
            prod = work.tile([P, fr], mybir.dt.float32, tag="prod")
            nc.vector.tensor_mul(prod, logp, em_v)
            scr = work.tile([P, fr], mybir.dt.float32, tag="scr")
            nc.vector.scalar_tensor_tensor(
                out=scr, in0=lab_v, scalar=1.0, in1=prod,
                op0=mybir.AluOpType.is_equal, op1=mybir.AluOpType.mult,
                accum_out=acc[:, acc_col:acc_col + 1],
            )

        # 7 blocks of 2 MiB per tensor; compute in [P, 2048] half-chains
        for b in range(B2 - 1):
            pred_t = ins.tile([P, BFREE], mybir.dt.float32, tag="pred")
            lab_t = ins.tile([P, BFREE], mybir.dt.int32, tag="lab")
            em_t = ins.tile([P, BFREE], mybir.dt.float32, tag="em")
            nc.sync.dma_start(out=pred_t, in_=pred_d[b])
            nc.sync.dma_start(out=lab_t, in_=lab_d[b])
            nc.sync.dma_start(out=em_t, in_=em_d[b])
            for h in range(2):
                sl = slice(h * HFREE, (h + 1) * HFREE)
                compute(pred_t[:, sl], lab_t[:, sl], em_t[:, sl],
                        HFREE, 2 * b + h)

        # last block: 4 chunks with own (deeper) slots to keep the DMA
        # stream dense through the tail
        b = B2 - 1
        Q = BFREE // NTAIL
        for q in range(NTAIL):
            sl = slice(q * Q, (q + 1) * Q)
            pred_q = ins.tile([P, Q], mybir.dt.float32, tag="predq", bufs=4)
            lab_q = ins.tile([P, Q], mybir.dt.int32, tag="labq", bufs=4)
            em_q = ins.tile([P, Q], mybir.dt.float32, tag="emq", bufs=4)
            nc.sync.dma_start(out=pred_q, in_=pred_d[b, :, sl])
            nc.sync.dma_start(out=lab_q, in_=lab_d[b, :, sl])
            nc.sync.dma_start(out=em_q, in_=em_d[b, :, sl])
            compute(pred_q, lab_q, em_q, Q, 2 * (B2 - 1) + q)

        nc.sync.dma_start(out=out_d, in_=acc)

    nc.compile()
    return nc


def _get_nc() -> bass.Bass:
    if not _nc_cache:
        _nc_cache.append(_build_nc())
    return _nc_cache[0]


def _weight_map(Wl, label_sum):
    """[P, NACC] weight map matching the flat-block accumulator layout:
    within any block column, partitions 0..63 hold the even class and
    64..127 the odd class."""
    denom = np.sum(label_sum, dtype=np.float32)
    wl = (-Wl / denom).astype(np.float32)
    cols = []
    for b in range(B2 - 1):
        pair = np.concatenate([np.full(P // 2, wl[2 * b]),
                               np.full(P // 2, wl[2 * b + 1])])
        cols.extend([pair, pair])  # one column per half-chain
    tail = np.concatenate([np.full(P // 2, wl[C - 2]),
                           np.full(P // 2, wl[C - 1])])
    cols.extend([tail] * NTAIL)
    return np.stack(cols, axis=1).astype(np.float32)


def _run(pred, label, Wl, label_sum, existmap, **spmd_kwargs):
    pred = np.ascontiguousarray(np.asarray(pred, dtype=np.float32))
    label = np.ascontiguousarray(np.asarray(label, dtype=np.int32))
    Wl = np.asarray(Wl, dtype=np.float32)
    label_sum = np.asarray(label_sum, dtype=np.float32)
    existmap = np.ascontiguousarray(np.asarray(existmap, dtype=np.float32))

    wl_map = _weight_map(Wl, label_sum)

    in_maps = []
    for i in range(N_CORES):
        in_maps.append({
            "pred": pred[i].reshape(B2, P, BFREE),
            "label": label[i].reshape(B2, P, BFREE),
            "existmap": existmap[i].reshape(B2, P, BFREE),
        })

    nc = _get_nc()
    res = bass_utils.run_bass_kernel_spmd(
        nc, in_maps, core_ids=list(range(N_CORES)), **spmd_kwargs)

    total = np.float32(0.0)
    for r in res.results:
        total += np.float32((r["out"] * wl_map).sum(dtype=np.float32))
    return np.array(total, dtype=np.float32), res


def kernel(pred, label, Wl, label_sum, existmap):
    out, _ = _run(pred, label, Wl, label_sum, existmap)
    return out


# revision 32
# speedup vs baseline: 1.2265x; 1.0056x over previous
"""Trainium2 Bass kernel for nn_ExistLCross (masked weighted -log loss).

reference:
    mask = (label == 1)
    per_elem = -log(pred + 0.01) * existmap * mask
    loss = einsum('nchw,c->', per_elem, Wl) / sum(label_sum)

Sharding: data-parallel over batch N=8 -> one batch item per NeuronCore.
Each core views its [16, 512, 512] shard as 8 flat 2-class blocks of
[128, 4096] (16 KB contiguous per partition -> ideal 2 MiB DMAs). Within a
block, partitions 0..63 hold the even class and 64..127 the odd class, so
per-partition sums separate the classes for free. Per block:
    ACT : logp = Ln(pred + 0.01)               (bias via ACT-made eps tile)
    DVE : logp *= existmap                     (tensor_tensor, in place)
    DVE : logp = (label == 1) * logp           (scalar_tensor_tensor, in
          place) with accum_out -> acc[:, col] (per-partition sums)
The final block runs as 8 eighth-chunks to shorten the post-DMA tail.
Each core DMAs acc [128, 15] back; the host applies the per-class weights
-Wl/sum(label_sum) via a [128, 15] weight map and adds up the 8 per-core
partials (the all-reduce).
"""

import sys
import types
from contextlib import ExitStack

import numpy as np

import concourse.bacc as bacc
import concourse.bass as bass
import concourse.tile as tile
from concourse import bass_utils, mybir

# This container's antenv lacks axon_hooks; bass_utils imports it whenever
# tracing is requested (e.g. via BASS_TRACE in the environment). Provide a
# no-op implementation so tracing degrades gracefully instead of raising.
if "antenv.axon_hooks" not in sys.modules:
    _hooks = types.ModuleType("antenv.axon_hooks")
    _hooks._hook = None
    _hooks.set_axon_ntff_profile_hook = lambda h: setattr(_hooks, "_hook", h)
    _hooks.get_axon_ntff_profile_hook = lambda: _hooks._hook
    sys.modules["antenv.axon_hooks"] = _hooks

N, C, H, W = 8, 16, 512, 512
P = 128
B2 = C // 2          # 8 two-class blocks
BFREE = 2 * H * W // P  # 4096 elements per partition per block
HFREE = BFREE // 2
EPS = 0.01
N_CORES = 8
NTAIL = 4            # last block in 4 chunks
NACC = 2 * (B2 - 1) + NTAIL  # 18 accumulator columns (one per half-chain)

_nc_cache = []


def _build_nc() -> bass.Bass:
    nc = bacc.Bacc("TRN2", target_bir_lowering=False, debug=False,
                   num_devices=N_CORES)

    pred_d = nc.dram_tensor("pred", [B2, P, BFREE], mybir.dt.float32,
                            kind="ExternalInput").ap()
    lab_d = nc.dram_tensor("label", [B2, P, BFREE], mybir.dt.int32,
                           kind="ExternalInput").ap()
    em_d = nc.dram_tensor("existmap", [B2, P, BFREE], mybir.dt.float32,
                          kind="ExternalInput").ap()
    out_d = nc.dram_tensor("out", [P, NACC], mybir.dt.float32,
                           kind="ExternalOutput").ap()

    with tile.TileContext(nc) as tc, ExitStack() as ctx:
        ins = ctx.enter_context(tc.tile_pool(name="ins", bufs=2))
        work = ctx.enter_context(tc.tile_pool(name="work", bufs=2))
        singles = ctx.enter_context(tc.tile_pool(name="singles", bufs=1))

        acc = singles.tile([P, NACC], mybir.dt.float32)
        ones = singles.tile([P, 1], mybir.dt.float32)
        eps_t = singles.tile([P, 1], mybir.dt.float32)
        nc.vector.memset(ones, 1.0)
        # eps = ones*0 + EPS, produced on ACT so every Ln below depends on
        # it same-engine (no cross-engine wait, no pre-Tile barrier).
        nc.scalar.activation(eps_t, ones, mybir.ActivationFunctionType.Copy,
                             bias=EPS, scale=0.0)

        def compute(pred_v, lab_v, em_v, fr, acc_col):
            """log/mask chain on [P, fr] SBUF views, accumulating the
            per-partition sums into acc[:, acc_col]."""
            logp = work.tile([P, fr], mybir.dt.float32, tag="logp")
            nc.scalar.activation(logp, pred_v,
                                 mybir.ActivationFunctionType.Ln, bias=eps_t)
            prod = work.tile([P, fr], mybir.dt.float32, tag="prod")
            nc.vector.tensor_mul(prod, logp, em_v)
            scr = work.tile([P, fr], mybir.dt.float32, tag="scr")
            nc.vector.scalar_tensor_tensor(
                out=scr, in0=lab_v, scalar=1.0, in1=prod,
                op0=mybir.AluOpType.is_equal, op1=mybir.AluOpType.mult,
                accum_out=acc[:, acc_col:acc_col + 1],
            )

        # 7 blocks of 2 MiB per tensor; compute in [P, 2048] half-chains
        for b in range(B2 - 1):
            pred_t = ins.tile([P, BFREE], mybir.dt.float32, tag="pred")
            lab_t = ins.tile([P, BFREE], mybir.dt.int32, tag="lab")
            em_t = ins.tile([P, BFREE], mybir.dt.float32, tag="em")
            nc.sync.dma_start(out=pred_t, in_=pred_d[b])
            nc.sync.dma_start(out=lab_t, in_=lab_d[b])
            nc.sync.dma_start(out=em_t, in_=em_d[b])
            for h in range(2):
                sl = slice(h * HFREE, (h + 1) * HFREE)
                compute(pred_t[:, sl], lab_t[:, sl], em_t[:, sl],
                        HFREE, 2 * b + h)

        # last block: tapered chunks so the final exposed DVE pair is
        # small; the big first chunk rides the block tags, the small ones
        # get their own deeper slots to keep the tail DMA stream dense
        b = B2 - 1
        off = 0
        for q, fr in enumerate((2048, 1024, 512, 512)):
            sl = slice(off, off + fr)
            off += fr
            if q == 0:
                pred_q = ins.tile([P, fr], mybir.dt.float32, tag="pred")
                lab_q = ins.tile([P, fr], mybir.dt.int32, tag="lab")
                em_q = ins.tile([P, fr], mybir.dt.float32, tag="em")
            else:
                pred_q = ins.tile([P, 1024], mybir.dt.float32, tag="predq",
                                  bufs=3, name="pred_q")[:, :fr]
                lab_q = ins.tile([P, 1024], mybir.dt.int32, tag="labq",
                                 bufs=3, name="lab_q")[:, :fr]
                em_q = ins.tile([P, 1024], mybir.dt.float32, tag="emq",
                                bufs=3, name="em_q")[:, :fr]
            nc.sync.dma_start(out=pred_q, in_=pred_d[b, :, sl])
            nc.sync.dma_start(out=lab_q, in_=lab_d[b, :, sl])
            nc.sync.dma_start(out=em_q, in_=em_d[b, :, sl])
            compute(pred_q, lab_q, em_q, fr, 2 * (B2 - 1) + q)

        nc.sync.dma_start(out=out_d, in_=acc)

    nc.compile()
    return nc


def _get_nc() -> bass.Bass:
    if not _nc_cache:
        _nc_cache.append(_build_nc())
    return _nc_cache[0]


def _weight_map(Wl, label_sum):
    """[P, NACC] weight map matching the flat-block accumulator layout:
    within any block column, partitions 0..63 hold the even class and
    64..127 the odd class."""
    denom = np.sum(label_sum, dtype=np.float32)
    wl = (-Wl / denom).astype(np.float32)
    cols = []
    for b in range(B2 - 1):
        pair = np.concatenate([np.full(P // 2, wl[2 * b]),
                               np.full(P // 2, wl[2 * b + 1])])
        cols.extend([pair, pair])  # one column per half-chain
    tail = np.concatenate([np.full(P // 2, wl[C - 2]),
                           np.full(P // 2, wl[C - 1])])
    cols.extend([tail] * NTAIL)
    return np.stack(cols, axis=1).astype(np.float32)


def _run(pred, label, Wl, label_sum, existmap, **spmd_kwargs):
    pred = np.ascontiguousarray(np.asarray(pred, dtype=np.float32))
    label = np.ascontiguousarray(np.asarray(label, dtype=np.int32))
    Wl = np.asarray(Wl, dtype=np.float32)
    label_sum = np.asarray(label_sum, dtype=np.float32)
    existmap = np.ascontiguousarray(np.asarray(existmap, dtype=np.float32))

    wl_map = _weight_map(Wl, label_sum)

    in_maps = []
    for i in range(N_CORES):
        in_maps.append({
            "pred": pred[i].reshape(B2, P, BFREE),
            "label": label[i].reshape(B2, P, BFREE),
            "existmap": existmap[i].reshape(B2, P, BFREE),
        })

    nc = _get_nc()
    res = bass_utils.run_bass_kernel_spmd(
        nc, in_maps, core_ids=list(range(N_CORES)), **spmd_kwargs)

    total = np.float32(0.0)
    for r in res.results:
        total += np.float32((r["out"] * wl_map).sum(dtype=np.float32))
    return np.array(total, dtype=np.float32), res


def kernel(pred, label, Wl, label_sum, existmap):
    out, _ = _run(pred, label, Wl, label_sum, existmap)
    return out
